# revision 10
# baseline (speedup 1.0000x reference)
"""HGAT message-passing kernel for Trainium2 (8 NeuronCores, SPMD).

Reference computation (B=4, N=4096, C_IN=128, C_OUT=64):
    h   = node_rep @ proj_W.T + proj_b                    # [B,N,64]
    f1  = rowsum(h * k_W[node_type]) + k_b[node_type]     # [B,N]
    f2  = rowsum(h * v_W[node_type]) + v_b[node_type]     # [B,N]
    L   = adj[i,j] * (f1[i] + f2[j])
    u   = sigmoid(L) - 0.5
    P   = softmax(u, axis=i)      # normalized over rows i, per column j
    out = P @ h                   # contract over j

Key algebra used on device:
  * softmax-over-i / contract-over-j means out = E @ (h / colsum) with
    E[i,j] = exp(sigmoid(L)) and colsum[j] = sum_i E[i,j]; the -0.5 and the
    softmax max-subtraction cancel in the ratio.
  * sigmoid(x) = 0.5 + 0.5*tanh(0.5 x); tanh and exp share one ACT table set.
  * exp's accum_out produces colsum for free.
  * The contraction is computed transposed: outT[c, i] += g[j, c]^T @ E[j, i]
    with g = h/colsum stationary (one 128x64 weight load per j-tile) and E
    streaming 512-wide — f32r matmuls with moving dim >= 256 run at full PE
    rate, vs 1/4 rate with a 64-wide moving dim.

Sharding: core c handles batch b=c//2 and j-half h=c%2 (rows of adj.T).
Host pre-transposes adj (so the device contracts over j on the partition
axis), gathers k_W/v_W rows by node_type (pure data movement), and sums the
two per-batch partial outputs at the end.
"""

import os
import sys

import ml_dtypes
import numpy as np

sys.path.insert(0, "/opt/trn_rl_repo")

import concourse.bass as bass  # noqa: E402
import concourse.tile as tile  # noqa: E402
from concourse import bacc  # noqa: E402
from concourse import mybir  # noqa: E402
from concourse.bass_utils import run_bass_kernel_spmd  # noqa: E402

B = 4
N = 4096
CIN = 128
COUT = 64
P = 128                      # SBUF partitions
NJ = N // 2                  # j rows per core (adjacency half)
NJT = NJ // P                # 16 j-tiles per core
NIC = N // 512               # 8 i-chunks of 512
MMF = 512                    # matmul moving-dim chunk (one PSUM bank)

F32 = mybir.dt.float32
BF16 = mybir.dt.bfloat16
AF = mybir.ActivationFunctionType
ALU = mybir.AluOpType

# dtype for the attention tensor + h operand of the final matmul.
# bf16 keeps the PE at full rate even when its clock is not ramped (f32r
# with a cold PE runs ~3x slower), and halves the et SBUF footprint.
ET_DTYPE = BF16

LAST_EXEC_NS = None
LAST_RESULTS = None


def build_nc(n=N, nj=NJ, et_dtype=None):
    """Build the single-core SPMD Bass program (same program on all cores)."""
    if et_dtype is None:
        et_dtype = ET_DTYPE
    # fp32 path: produce matmul operands as float32r (TF32-like). The
    # verifier requires producers to round to f32r.
    mm_dtype = mybir.dt.float32r if et_dtype == F32 else et_dtype
    njt = nj // P
    nic = n // 512

    nc = bacc.Bacc()
    adjt_d = nc.dram_tensor("adjt", [nj, n], BF16, kind="ExternalInput")
    xt_d = nc.dram_tensor("xt", [CIN, n], F32, kind="ExternalInput")
    xth_d = nc.dram_tensor("xth", [CIN, nj], F32, kind="ExternalInput")
    wpt_d = nc.dram_tensor("wpt", [CIN, COUT], F32, kind="ExternalInput")
    bpcol_d = nc.dram_tensor("bpcol", [COUT, 1], F32, kind="ExternalInput")
    bpb_d = nc.dram_tensor("bpb", [P, COUT], F32, kind="ExternalInput")
    kwt_d = nc.dram_tensor("kwt", [COUT, n], F32, kind="ExternalInput")
    kbrow_d = nc.dram_tensor("kbrow", [1, n], F32, kind="ExternalInput")
    vwn_d = nc.dram_tensor("vwn", [P, njt * COUT], F32, kind="ExternalInput")
    vbcol_d = nc.dram_tensor("vbcol", [P, njt], F32, kind="ExternalInput")
    outp_d = nc.dram_tensor("outp", [COUT, n], F32, kind="ExternalOutput")

    with tile.TileContext(nc) as tc:
        with (
            tc.tile_pool(name="adjp", bufs=3) as adjp,
            tc.tile_pool(name="workp", bufs=2) as workp,
            tc.tile_pool(name="etp", bufs=3) as etp,
            tc.tile_pool(name="singles", bufs=1) as singles,
            tc.tile_pool(name="smalls", bufs=3) as smalls,
            tc.tile_pool(name="stream", bufs=2) as stream,
            tc.tile_pool(name="dscratch", bufs=1, space="DRAM") as dscratch,
        ):
            # ---------------- small parameter loads ----------------
            wpt_s = singles.tile([CIN, COUT], F32)
            nc.sync.dma_start(wpt_s, wpt_d[:, :])
            bpcol_s = singles.tile([COUT, 1], F32)
            nc.sync.dma_start(bpcol_s, bpcol_d[:, :])
            bpb_s = singles.tile([P, COUT], F32)
            nc.sync.dma_start(bpb_s, bpb_d[:, :])
            vbcol_s = singles.tile([P, njt], F32)
            nc.sync.dma_start(vbcol_s, vbcol_d[:, :])

            ones64 = singles.tile([COUT, 1], F32)
            nc.vector.memset(ones64, 1.0)
            zero_col = singles.tile([P, 1], F32)
            nc.vector.memset(zero_col, 0.0)
            half_col = singles.tile([P, 1], F32)
            nc.vector.memset(half_col, 0.5)

            f1s = dscratch.tile([1, n], F32)
            hn = singles.tile([P, njt * COUT], F32)
            f2c = singles.tile([P, njt], F32)

            with tc.tile_pool(name="psA", bufs=2, space="PSUM") as psA:
                # ------- f1 row, streamed in 512-col chunks -------
                # f1[i] = sum_o (x@Wp.T + bp)[i,o] * KW[i,o] + kb[i]
                for ic in range(nic):
                    sl = slice(ic * 512, (ic + 1) * 512)
                    xtc = stream.tile([CIN, 512], F32, tag="xtc")
                    nc.sync.dma_start(xtc, xt_d[:, sl])
                    psh = psA.tile([COUT, 512], F32, tag="ps")
                    nc.tensor.matmul(psh, lhsT=wpt_s, rhs=xtc, start=True, stop=True)
                    hTc = stream.tile([COUT, 512], F32, tag="hTc")
                    nc.vector.tensor_scalar_add(hTc, psh, bpcol_s)
                    kwc = stream.tile([COUT, 512], F32, tag="kwc")
                    nc.sync.dma_start(kwc, kwt_d[:, sl])
                    nc.vector.tensor_mul(hTc, hTc, kwc)
                    psf = psA.tile([1, 512], F32, tag="ps", padded_shape=[128, 512])
                    nc.tensor.matmul(psf, lhsT=ones64, rhs=hTc, start=True, stop=True)
                    kbc = stream.tile([1, 512], F32, tag="kbc")
                    nc.sync.dma_start(kbc, kbrow_d[:, sl])
                    f1rc = stream.tile([1, 512], F32, tag="f1rc")
                    nc.vector.tensor_add(f1rc, psf, kbc)
                    nc.sync.dma_start(f1s[:, sl], f1rc)

                # ------- h natural (j-half nodes) for f2 and g -------
                for t in range(njt):
                    osl = slice(t * COUT, (t + 1) * COUT)
                    xthc = stream.tile([CIN, P], F32, tag="xthc")
                    nc.sync.dma_start(xthc, xth_d[:, t * P:(t + 1) * P])
                    psn = psA.tile([P, COUT], F32, tag="ps", padded_shape=[128, 512])
                    nc.tensor.matmul(psn, lhsT=xthc, rhs=wpt_s, start=True, stop=True)
                    nc.vector.tensor_add(hn[:, osl], psn, bpb_s)
                    vwc = stream.tile([P, COUT], F32, tag="vwc")
                    nc.sync.dma_start(vwc, vwn_d[:, osl])
                    pvc = stream.tile([P, COUT], F32, tag="pvc")
                    nc.vector.tensor_mul(pvc, hn[:, osl], vwc)
                    nc.vector.tensor_reduce(
                        f2c[:, t:t + 1], pvc, axis=mybir.AxisListType.X, op=ALU.add
                    )
            f2cb = singles.tile([P, njt], F32)
            nc.vector.tensor_add(f2cb, f2c, vbcol_s)

            # broadcast f1 across all 128 partitions via DRAM round-trip
            f1b = singles.tile([P, n], F32)
            f1s_bcast = bass.AP(tensor=f1s.tensor, offset=f1s.offset, ap=[[0, P], [1, n]])
            nc.sync.dma_start(f1b, f1s_bcast)

            # ---------------- main loop over j-tiles ----------------
            # outT[c, i] accumulated in PSUM [COUT, n] = 8 banks; each 512-col
            # bank is one accumulation group: start on jt==0, stop on the last.
            # Software-pipelined: stt for tile jt+1 is issued on the DVE
            # BEFORE reciprocal(jt), which blocks on exp(jt)'s accumulator —
            # otherwise the DVE FIFO serializes the whole stt→tanh→exp chain.
            def stt_tile(jt):
                adjt_t = adjp.tile([P, n], BF16, tag="adj")
                nc.sync.dma_start(adjt_t, adjt_d[jt * P:(jt + 1) * P, :])
                lt = workp.tile([P, n], F32, tag="L")
                # L[j,i] = (f1[i] + f2[j]) * adjT[j,i] — one fused DVE pass
                nc.vector.scalar_tensor_tensor(
                    lt, f1b, f2cb[:, jt:jt + 1], adjt_t,
                    op0=ALU.add, op1=ALU.mult,
                )
                return lt

            with tc.tile_pool(name="psO", bufs=1, space="PSUM") as psO:
                ps_out = psO.tile([COUT, n], F32)

                lt_cur = stt_tile(0)
                for jt in range(njt):
                    # t = tanh(L/2) in place;  E = exp(t/2 + 1/2) = exp(sigmoid(L))
                    nc.scalar.activation(lt_cur, lt_cur, AF.Tanh, bias=zero_col, scale=0.5)
                    et = etp.tile([P, n], mm_dtype, tag="et")
                    cs = smalls.tile([P, 1], F32, tag="cs")
                    nc.scalar.activation(
                        et, lt_cur, AF.Exp, bias=half_col, scale=0.5, accum_out=cs
                    )

                    if jt + 1 < njt:
                        lt_next = stt_tile(jt + 1)

                    rc = smalls.tile([P, 1], F32, tag="rc")
                    nc.vector.reciprocal(rc, cs)
                    g = smalls.tile([P, COUT], mm_dtype, tag="g")
                    nc.vector.tensor_scalar_mul(g, hn[:, jt * COUT:(jt + 1) * COUT], rc)

                    for k in range(n // MMF):
                        nc.tensor.matmul(
                            ps_out[:, k * MMF:(k + 1) * MMF],
                            lhsT=g,
                            rhs=et[:, k * MMF:(k + 1) * MMF],
                            start=(jt == 0),
                            stop=(jt == njt - 1),
                        )
                    lt_cur = lt_next

                # evacuate PSUM -> SBUF -> DRAM in 4 chunks; alternate DVE/ACT
                for c in range(4):
                    sl = slice(c * 1024, (c + 1) * 1024)
                    ob = stream.tile([COUT, 1024], F32, tag="ob")
                    if c % 2 == 0:
                        nc.vector.tensor_copy(ob, ps_out[:, sl])
                    else:
                        nc.scalar.copy(ob, ps_out[:, sl])
                    nc.sync.dma_start(outp_d[:, sl], ob)

    nc.finalize()
    return nc


def _prep_in_maps(node_rep, adj_matrix, node_type, proj_W, proj_b, k_W, k_b, v_W, v_b):
    """Host-side shard prep (data movement / layout only, no FLOPs on the model math)."""
    f32 = np.float32
    node_rep = np.ascontiguousarray(np.asarray(node_rep, dtype=f32))
    adj = np.ascontiguousarray(np.asarray(adj_matrix, dtype=f32))
    nt = np.asarray(node_type).astype(np.int64) % 5
    proj_W = np.asarray(proj_W, dtype=f32)
    proj_b = np.asarray(proj_b, dtype=f32)
    k_W = np.asarray(k_W, dtype=f32)
    k_b = np.asarray(k_b, dtype=f32)
    v_W = np.asarray(v_W, dtype=f32)
    v_b = np.asarray(v_b, dtype=f32)

    adjT = np.ascontiguousarray(adj.T.astype(ml_dtypes.bfloat16))  # adjT[j, i]
    wpt = np.ascontiguousarray(proj_W.T)                    # [CIN, COUT]
    bpcol = np.ascontiguousarray(proj_b[:, None])           # [COUT, 1]
    bpb = np.ascontiguousarray(np.broadcast_to(proj_b[None, :], (P, COUT)))
    KW = k_W[nt]                                            # [N, COUT] gather
    kwt = np.ascontiguousarray(KW.T)                        # [COUT, N]
    kbrow = np.ascontiguousarray(k_b[nt][None, :])          # [1, N]
    VW = v_W[nt]                                            # [N, COUT]
    vb = v_b[nt]                                            # [N]

    in_maps = []
    for core in range(8):
        b, half = divmod(core, 2)
        jsl = slice(half * NJ, (half + 1) * NJ)
        xT = np.ascontiguousarray(node_rep[b].T)            # [CIN, N]
        vw_h = VW[jsl]                                      # [NJ, COUT]
        vwn = np.ascontiguousarray(
            vw_h.reshape(NJT, P, COUT).transpose(1, 0, 2).reshape(P, NJT * COUT)
        )
        vbcol = np.ascontiguousarray(vb[jsl].reshape(NJT, P).T)  # [P, NJT]
        in_maps.append({
            "adjt": np.ascontiguousarray(adjT[jsl, :]),
            "xt": xT,
            "xth": np.ascontiguousarray(xT[:, jsl]),
            "wpt": wpt,
            "bpcol": bpcol,
            "bpb": bpb,
            "kwt": kwt,
            "kbrow": kbrow,
            "vwn": vwn,
            "vbcol": vbcol,
        })
    return in_maps


def kernel(node_rep, adj_matrix, node_type, proj_W, proj_b, k_W, k_b, v_W, v_b):
    global LAST_EXEC_NS, LAST_RESULTS
    in_maps = _prep_in_maps(
        node_rep, adj_matrix, node_type, proj_W, proj_b, k_W, k_b, v_W, v_b
    )
    nc = build_nc()
    trace = os.environ.get("KERNEL_TRACE", "0") == "1"
    res = run_bass_kernel_spmd(nc, in_maps, core_ids=list(range(8)), trace=trace)
    LAST_EXEC_NS = res.exec_time_ns
    LAST_RESULTS = res

    out = np.empty((B, N, COUT), dtype=np.float32)
    for b in range(B):
        p0 = np.asarray(res.results[2 * b]["outp"], dtype=np.float32)
        p1 = np.asarray(res.results[2 * b + 1]["outp"], dtype=np.float32)
        out[b] = (p0 + p1).T
    return out
